# revision 9
# baseline (speedup 1.0000x reference)
"""Multi-head attention (RoPE on k/v) Bass kernel for 8 TRN2 NeuronCores.

Sharding: tensor-parallel over heads (2 heads/core, both batches) for the
QKV projections + attention; one AllToAll redistributes context to a
row-sharded output projection. All matmuls fp16.

v2: scores matmul pairs run concurrently in disjoint PE row-groups
(tile_position (0,0)/(64,0)); softmax exp is split across the Scalar
engine (exact LUT exp) and Vector/GpSimd (one-instruction Schraudolph
exp: round(A*s+B) -> int16, bit-identical to an fp16 exponential
approximation, max rel err ~2%).

Self-contained: hardcodes shapes from the problem spec.
"""
import os
import sys
import types

import numpy as np


def _install_ntff_hook():
    """antenv.axon_hooks is missing from this image; synthesize it so
    run_bass_kernel_spmd(trace=True) works. Harmless when trace=False."""
    if "antenv.axon_hooks" in sys.modules:
        return
    try:
        from trn_agent_boot.trn_boot import _ntff_profile_via_ctypes

        hook = _ntff_profile_via_ctypes("/opt/axon/libaxon_pjrt.so")
    except Exception:
        hook = None
    mod = types.ModuleType("antenv.axon_hooks")
    mod._hook = hook
    mod.get_axon_ntff_profile_hook = lambda: mod._hook
    mod.set_axon_ntff_profile_hook = lambda h: setattr(mod, "_hook", h)
    sys.modules["antenv.axon_hooks"] = mod
    try:
        import antenv

        antenv.axon_hooks = mod
    except Exception:
        pass


_install_ntff_hook()

import concourse.bass as bass  # noqa: E402
import concourse.mybir as mybir  # noqa: E402
import concourse.tile as tile  # noqa: E402
from concourse import bacc  # noqa: E402
from concourse.bass import ds  # noqa: E402
from concourse.bass_utils import run_bass_kernel_spmd  # noqa: E402

B, S, D, H = 2, 2048, 1024, 16
R = B * S              # 4096 flattened rows
NC = 8                 # cores
HPC = H // NC          # 2 heads per core
CW = D // NC           # 128 ctx cols per core
DH = D // H            # 64 head dim
RW = R // NC           # 512 output rows per core
F32 = mybir.dt.float32
F16 = mybir.dt.float16
I16 = mybir.dt.int16
FP = np.float32

# Schraudolph fp16 exp constants: i16 = round(raw_score * A + B);
# bitcast(i16) ~= exp(raw_score / 8).
A_SCH = float(1024.0 * np.log2(np.e) / 8.0)
B_SCH = float(15.0 * 1024.0 - 0.043677448 * 1024.0)

# exp engine per kt (16 key tiles per q tile): a=Scalar(exact LUT),
# v=Vector(Schraudolph).  GpSimd cannot read PSUM, so it gets no slots.
EXP_ENG = "avavavavavavavaa"


def _perm_local() -> np.ndarray:
    """Within a core's 128-col slice: [A-evens, A-odds, B-evens, B-odds]."""
    a_ev = np.arange(0, 64, 2)
    a_od = np.arange(1, 64, 2)
    return np.concatenate([a_ev, a_od, 64 + a_ev, 64 + a_od])


def _build_program():
    nc = bacc.Bacc("TRN2", target_bir_lowering=False, debug=False, num_devices=NC)

    # ---- external I/O ----
    xq_d = nc.dram_tensor("xq", [D, R], F16, kind="ExternalInput").ap()
    xk_d = nc.dram_tensor("xk", [D, R], F16, kind="ExternalInput").ap()
    xv_d = nc.dram_tensor("xv", [D, R], F16, kind="ExternalInput").ap()
    wq_d = nc.dram_tensor("wq", [D, CW], F16, kind="ExternalInput").ap()
    wk_d = nc.dram_tensor("wk", [D, CW], F16, kind="ExternalInput").ap()
    wv_d = nc.dram_tensor("wv", [D, CW], F16, kind="ExternalInput").ap()
    wp_d = nc.dram_tensor("wp", [D, D], F16, kind="ExternalInput").ap()
    cs_d = nc.dram_tensor("cs", [128, R], F16, kind="ExternalInput").ap()
    sn_d = nc.dram_tensor("sn", [128, R], F16, kind="ExternalInput").ap()
    sw_d = nc.dram_tensor("sw", [128, 128], F16, kind="ExternalInput").ap()
    id_d = nc.dram_tensor("ident", [128, 128], F16, kind="ExternalInput").ap()
    on_d = nc.dram_tensor("ones", [128, 64], F16, kind="ExternalInput").ap()
    bi_d = nc.dram_tensor("bias", [128, D], F32, kind="ExternalInput").ap()
    out_d = nc.dram_tensor("out", [RW, D], F32, kind="ExternalOutput").ap()

    KT = D // 128   # 8 contraction tiles for projections
    RT = R // 512   # 8 row tiles
    QT = S // 512   # 4 q tiles per batch
    ST = S // 128   # 16 k tiles per batch

    with tile.TileContext(nc) as tc:
        with (
            tc.tile_pool(name="const", bufs=1) as const,
            tc.tile_pool(name="persist", bufs=1) as persist,
            tc.tile_pool(name="dram", bufs=1, space="DRAM") as dram,
            tc.tile_pool(name="wqkv", bufs=1) as wpool,
            tc.tile_pool(name="trig", bufs=1) as trig,
            tc.tile_pool(name="xin", bufs=3) as xin,
            tc.tile_pool(name="vtmp", bufs=1) as vtmp,
            tc.tile_pool(name="rope", bufs=3) as rp,
            tc.tile_pool(name="epool", bufs=1) as ep,
            tc.tile_pool(name="norm", bufs=1) as npl,
            tc.tile_pool(name="oio", bufs=2) as oio,
            tc.tile_pool(name="pp", bufs=1, space="PSUM") as pp,
            tc.tile_pool(name="spsum", bufs=1, space="PSUM") as sp,
            tc.tile_pool(name="cpsum", bufs=1, space="PSUM") as cp,
        ):
            # ---- constants / weights ----
            # Only wk + sw are DMA'd up front; the rest are issued at their
            # first use point so the opening k-proj isn't DMA-starved.
            wk_sb = wpool.tile([128, KT, CW], F16)
            nc.sync.dma_start(wk_sb[:], wk_d.rearrange("(kt p) m -> p kt m", p=128))
            sw_sb = const.tile([128, 128], F16)
            nc.sync.dma_start(sw_sb[:], sw_d[:])
            id_sb = const.tile([128, 128], F16)
            on_sb = const.tile([128, 64], F16)
            wv_sb = wpool.tile([128, KT, CW], F16)
            wq_sb = wpool.tile([128, KT, CW], F16)
            cs_sb = trig.tile([128, R], F16)
            sn_sb = trig.tile([128, R], F16)
            bi_sb = const.tile([128, D], F32)
            wp_sb = const.tile([128, KT, D], F16)

            qpT = persist.tile([128, R], F16)
            kpT = persist.tile([128, R], F16)
            vaug = persist.tile([128, R // 128, 256], F16)
            vpT = vtmp.tile([128, R], F16)

            def emit_vaug_init():
                nc.vector.tensor_copy(
                    vaug[:, :, 64:128],
                    on_sb[:, :, None].rearrange("p o n -> p n o").to_broadcast((128, R // 128, 64)),
                )
                nc.vector.tensor_copy(
                    vaug[:, :, 192:256],
                    on_sb[:, :, None].rearrange("p o n -> p n o").to_broadcast((128, R // 128, 64)),
                )

            _chunk_rows = [128, 128, 128, 64, 64]
            a2a_ins = [dram.tile([NC, 128, _chunk_rows[p]], F16,
                                 name=f"a2ain{p}", tag=f"a2ain{p}")
                       for p in range(5)]
            a2a_outs = [dram.tile([NC, 128, _chunk_rows[p]], F16,
                                  name=f"a2aout{p}", tag=f"a2aout{p}")
                        for p in range(5)]

            RTB = RT // B     # 4 row-tiles per batch
            CHUNKS = [(0, 2), (2, 4), (4, 6), (6, 7), (7, 8)]  # groups per A2A

            def prefetch_x(ti, rt):
                xd = (xk_d, xv_d, xq_d)[ti]
                rsl = ds(rt * 512, 512)
                x_sb = xin.tile([128, KT, 512], F16, tag="xin", name="x_sb")
                # split so the first contraction tiles land (and unblock the
                # matmuls) before the whole slice has arrived
                xr = xd.rearrange("(kt p) r -> p kt r", p=128)
                nc.sync.dma_start(x_sb[:, 0:2], xr[:, 0:2, rsl])
                nc.sync.dma_start(x_sb[:, 2:KT], xr[:, 2:KT, rsl])
                return x_sb

            def emit_proj_tile(ti, rt, x_sb=None):
                xd, wsb, dest, do_rope = (
                    (xk_d, wk_sb, kpT, True),
                    (xv_d, wv_sb, vpT, True),
                    (xq_d, wq_sb, qpT, False),
                )[ti]
                rsl = ds(rt * 512, 512)
                if x_sb is None:
                    x_sb = prefetch_x(ti, rt)
                ps = pp.tile([128, 512], F32, tag="proj", bufs=2, name="ps")
                for kt in range(KT):
                    nc.tensor.matmul(
                        ps[:], wsb[:, kt], x_sb[:, kt],
                        start=(kt == 0), stop=(kt == KT - 1),
                    )
                if not do_rope:
                    nc.vector.tensor_copy(dest[:, rsl], ps[:])
                else:
                    raw = rp.tile([128, 512], F16, tag="raw", name="raw")
                    nc.vector.tensor_copy(raw[:], ps[:])
                    sps = pp.tile([128, 512], F32, tag="proj", bufs=2, name="sps")
                    nc.tensor.matmul(sps[:], sw_sb[:], raw[:], start=True, stop=True)
                    t1 = rp.tile([128, 512], F32, tag="t1", name="t1")
                    nc.gpsimd.tensor_tensor(
                        t1[:], raw[:], cs_sb[:, rsl], mybir.AluOpType.mult)
                    t2 = rp.tile([128, 512], F32, tag="t2", name="t2")
                    nc.vector.tensor_tensor(
                        t2[:], sps[:], sn_sb[:, rsl], mybir.AluOpType.mult)
                    nc.vector.tensor_tensor(
                        dest[:, rsl], t1[:], t2[:], mybir.AluOpType.add)

            def emit_vtrans(ct):
                tpx = pp.tile([128, 512], F16, tag="proj", bufs=2, name="tpx")
                tp = tpx[:, 0:128]
                nc.tensor.transpose(tp, vpT[:, ds(ct * 128, 128)], id_sb[:])
                nc.vector.tensor_copy(vaug[:, ct, 0:64], tp[:, 0:64])
                nc.vector.tensor_copy(vaug[:, ct, 128:192], tp[:, 64:128])

            def emit_attn_qt(bb, qt):
                """Returns a deferred-outproj closure (or None): caller emits
                it after queueing more PE work, so the output-projection
                matmuls (which wait on the AllToAll) don't head-of-line-block
                the PE FIFO."""
                qsl = ds(bb * S + qt * 512, 512)
                # E[:, kt, 0, :] = head A weights, [:, kt, 1, :] = head B
                E = ep.tile([128, ST, 2, 512], F16, tag="E", bufs=2, name="E")
                cA = cp.tile([128, 512], F32, tag="cA", bufs=1, name="cA")
                cB = cp.tile([128, 512], F32, tag="cB", bufs=1, name="cB")

                def ctx_kt(kt):
                    ct = bb * ST + kt
                    nc.tensor.matmul(cA[:], vaug[:, ct, 0:128], E[:, kt, 0],
                                     start=(kt == 0), stop=(kt == ST - 1))
                    nc.tensor.matmul(cB[:], vaug[:, ct, 128:256], E[:, kt, 1],
                                     start=(kt == 0), stop=(kt == ST - 1))

                for kt in range(ST):
                    ksl = ds(bb * S + kt * 128, 128)
                    ps = sp.tile([128, 2, 512], F32, tag="sc", bufs=2,
                                 name="ps_sc")
                    nc.tensor.matmul(
                        ps[:, 0], kpT[0:64, ksl], qpT[0:64, qsl],
                        start=True, stop=True, tile_position=(0, 0))
                    nc.tensor.matmul(
                        ps[:, 1], kpT[64:128, ksl], qpT[64:128, qsl],
                        start=True, stop=True, tile_position=(64, 0))
                    eng = EXP_ENG[kt]
                    if eng == "a":
                        nc.scalar.activation(
                            E[:, kt], ps[:],
                            mybir.ActivationFunctionType.Exp, scale=0.125)
                    elif eng == "v":
                        nc.vector.tensor_scalar(
                            E[:, kt].bitcast(I16), ps[:], A_SCH, B_SCH,
                            mybir.AluOpType.mult, mybir.AluOpType.add)
                    else:
                        nc.gpsimd.tensor_scalar(
                            E[:, kt].bitcast(I16), ps[:], A_SCH, B_SCH,
                            mybir.AluOpType.mult, mybir.AluOpType.add)
                    if kt >= 2:
                        ctx_kt(kt - 2)
                ctx_kt(ST - 2)
                ctx_kt(ST - 1)

                zhi = npl.tile([128, 1024], F32, tag="zhi", name="zhi")
                nc.vector.tensor_copy(zhi[64:128, 0:512], cA[64:128])
                nc.vector.tensor_copy(zhi[64:128, 512:1024], cB[64:128])
                zlo = npl.tile([64, 1024], F32, tag="zlo", name="zlo")
                nc.sync.dma_start(zlo[:], zhi[64:128, :])
                zr = npl.tile([64, 1024], F32, tag="zr", name="zr")
                nc.vector.reciprocal_approx_fast(zr[:], zlo[:])
                ctxA = npl.tile([64, 512], F16, tag="ctxA", name="ctxA")
                nc.vector.tensor_tensor(
                    ctxA[:], cA[0:64], zr[:, 0:512], mybir.AluOpType.mult)
                ctxB = npl.tile([64, 512], F16, tag="ctxB", name="ctxB")
                nc.vector.tensor_tensor(
                    ctxB[:], cB[0:64], zr[:, 512:1024], mybir.AluOpType.mult)
                shard = bb * QT + qt
                ck = next(i for i, (a, b) in enumerate(CHUNKS) if a <= shard < b)
                a, b = CHUNKS[ck]
                rsl2 = ds((shard - a) * 64, 64)
                nc.sync.dma_start(
                    a2a_ins[ck][:, 0:64, rsl2].rearrange("j p r -> p j r"),
                    ctxA[:].rearrange("p (j r) -> p j r", j=NC))
                nc.sync.dma_start(
                    a2a_ins[ck][:, 64:128, rsl2].rearrange("j p r -> p j r"),
                    ctxB[:].rearrange("p (j r) -> p j r", j=NC))
                if shard != b - 1:
                    return None
                nc.gpsimd.collective_compute(
                    "AllToAll",
                    mybir.AluOpType.bypass,
                    replica_groups=[list(range(NC))],
                    ins=[a2a_ins[ck].opt()],
                    outs=[a2a_outs[ck].opt()],
                )
                nrows = (b - a) * 64
                lh = oio.tile([128, NC, 128], F16, tag="lh", name="lh")
                nc.sync.dma_start(
                    lh[:, :, 0:nrows],
                    a2a_outs[ck][:].rearrange("j p r -> p j r"),
                )

                def do_outproj():
                    for oc in range(2):
                        po = pp.tile([128, 512], F32, tag="proj", bufs=2,
                                     name="po")
                        for j in range(NC):
                            nc.tensor.matmul(
                                po[0:nrows], lh[:, j, 0:nrows],
                                wp_sb[:, j, ds(oc * 512, 512)],
                                start=(j == 0), stop=(j == NC - 1))
                        ob = oio.tile([128, 512], F32, tag="ob", name="ob")
                        nc.vector.tensor_tensor(
                            ob[0:nrows], po[0:nrows], bi_sb[0:nrows, ds(oc * 512, 512)],
                            mybir.AluOpType.add)
                        nc.sync.dma_start(
                            out_d[ds(a * 64, nrows), ds(oc * 512, 512)], ob[0:nrows])

                return do_outproj

            # ---- batch 0 projections ----
            # DMA order matters at startup: the first k-proj tile needs only
            # wk + x(0,0) (+ trig/sw for its rope) — everything else later.
            x00 = prefetch_x(0, 0)
            x01 = prefetch_x(0, 1)
            nc.sync.dma_start(cs_sb[:], cs_d[:])
            nc.sync.dma_start(sn_sb[:], sn_d[:])
            emit_proj_tile(0, 0, x00)
            emit_proj_tile(0, 1, x01)
            nc.sync.dma_start(wv_sb[:], wv_d.rearrange("(kt p) m -> p kt m", p=128))
            nc.sync.dma_start(wq_sb[:], wq_d.rearrange("(kt p) m -> p kt m", p=128))
            nc.sync.dma_start(id_sb[:], id_d[:])
            nc.sync.dma_start(on_sb[:], on_d[:])
            for rt in range(2, RTB):
                emit_proj_tile(0, rt)
            emit_vaug_init()
            for ti in (1, 2):
                for rt in range(0, RTB):
                    emit_proj_tile(ti, rt)
            nc.sync.dma_start(bi_sb[:], bi_d[:])
            nc.sync.dma_start(wp_sb[:], wp_d.rearrange("(kt p) o -> p kt o", p=128))
            for ct in range(0, S // 128):
                emit_vtrans(ct)
            # ---- batch 0 attention; batch 1 projections interleaved ----
            # piece order: k then v (so b1 vtrans can run right after qt3),
            # then q (only needed once b1 attention starts)
            b1_pieces = ([(0, rt) for rt in range(RTB, 2 * RTB)]
                         + [(1, rt) for rt in range(RTB, 2 * RTB)]
                         + [(2, rt) for rt in range(RTB, 2 * RTB)])
            pending = None
            for qt in range(QT):
                nxt = emit_attn_qt(0, qt)
                for ti, rt in b1_pieces[qt * 3:(qt + 1) * 3]:
                    emit_proj_tile(ti, rt)
                if pending is not None:
                    pending()
                pending = nxt
            # b1 vtrans burst overlaps the chunk-1 collective latency
            for ct in range(S // 128, 2 * (S // 128)):
                emit_vtrans(ct)
            if pending is not None:
                pending()
                pending = None
            for qt in range(QT):
                nxt = emit_attn_qt(1, qt)
                if pending is not None:
                    pending()
                pending = nxt
            if pending is not None:
                pending()

    nc.compile()
    return nc


_PROGRAM = None


def _get_program():
    global _PROGRAM
    if _PROGRAM is None:
        _PROGRAM = _build_program()
    return _PROGRAM


def _host_prep(q, k, v, Wq, Wk, Wv, Wp, bp):
    """Build the 8 per-core input maps."""
    rr = lambda a: np.ascontiguousarray(a, dtype=np.float32).astype(np.float16)
    xqT = rr(q.reshape(R, D).T)
    xkT = rr(k.reshape(R, D).T)
    xvT = rr(v.reshape(R, D).T)

    pl = _perm_local()
    perm_global = np.concatenate([128 * c + pl for c in range(NC)])
    wpT = rr(np.ascontiguousarray(Wp.T[perm_global, :]))

    # trig tables
    half = D // 2
    pos = np.arange(S, dtype=np.float64)
    theta = 1.0 / (10000.0 ** (2.0 * np.arange(half, dtype=np.float64) / D))
    ang = pos[:, None] * theta[None, :]          # [S, half]
    cosf = np.cos(ang).astype(FP)                # [S, half]
    sinf = np.sin(ang).astype(FP)

    sw = np.zeros((128, 128), np.float16)
    for m in range(128):
        p = (m + 32) % 64 + 64 * (m // 64)
        sw[p, m] = 1.0
    ident = np.eye(128, dtype=np.float16)
    ones = np.ones((128, 64), np.float16)
    bias = np.broadcast_to(bp.astype(FP), (128, D)).copy()

    in_maps = []
    for c in range(NC):
        cols = 128 * c + pl
        wq_c = rr(np.ascontiguousarray(Wq[cols, :].T))
        wk_c = rr(np.ascontiguousarray(Wk[cols, :].T))
        wv_c = rr(np.ascontiguousarray(Wv[cols, :].T))
        # pair index per partition p (see _perm_local ordering)
        j = np.empty(128, np.int64)
        j[0:32] = 64 * c + np.arange(32)
        j[32:64] = 64 * c + np.arange(32)
        j[64:96] = 64 * c + 32 + np.arange(32)
        j[96:128] = 64 * c + 32 + np.arange(32)
        cs1 = cosf[:, j].T                        # [128, S]
        sn1 = sinf[:, j].T.copy()
        sn1[0:32] *= -1.0
        sn1[64:96] *= -1.0
        cs = np.tile(cs1, (1, B)).astype(np.float16)      # [128, R]
        sn = np.tile(sn1, (1, B)).astype(np.float16)
        in_maps.append({
            "xq": xqT, "xk": xkT, "xv": xvT,
            "wq": wq_c, "wk": wk_c, "wv": wv_c,
            "wp": wpT, "cs": cs, "sn": sn,
            "sw": sw, "ident": ident, "ones": ones, "bias": bias,
        })
    return in_maps


def run(inputs, trace=False, trace_cores=None):
    nc = _get_program()
    in_maps = _host_prep(**inputs)
    res = run_bass_kernel_spmd(
        nc, in_maps, core_ids=list(range(NC)), trace=trace,
        trace_cores=trace_cores,
    )
    outs = np.stack([res.results[c]["out"] for c in range(NC)])  # [c, 512, D]
    # local row (128p + 64g' + i) on core c == global row 512*(2p+g') + 64c + i
    lo = outs.reshape(NC, NC, 64, D)              # [core, (2p,g'), i, D]
    full = lo.transpose(1, 0, 2, 3).reshape(B, S, D)
    return full, res


def kernel(**inputs) -> np.ndarray:
    trace = bool(int(os.environ.get("TRN_TRACE", "0")))
    full, res = run(inputs, trace=trace)
    if trace and res.exec_time_ns is not None:
        print(f"HW exec time: {res.exec_time_ns} ns")
    return full


# revision 12
# speedup vs baseline: 1.2273x; 1.2273x over previous
"""Multi-head attention (RoPE on k/v) Bass kernel for 8 TRN2 NeuronCores.

Sharding: tensor-parallel over heads (2 heads/core, both batches) for the
QKV projections + attention; one AllToAll redistributes context to a
row-sharded output projection. All matmuls fp16.

v2: scores matmul pairs run concurrently in disjoint PE row-groups
(tile_position (0,0)/(64,0)); softmax exp is split across the Scalar
engine (exact LUT exp) and Vector/GpSimd (one-instruction Schraudolph
exp: round(A*s+B) -> int16, bit-identical to an fp16 exponential
approximation, max rel err ~2%).

Self-contained: hardcodes shapes from the problem spec.
"""
import os
import sys
import types

import numpy as np


def _install_ntff_hook():
    """antenv.axon_hooks is missing from this image; synthesize it so
    run_bass_kernel_spmd(trace=True) works. Harmless when trace=False."""
    if "antenv.axon_hooks" in sys.modules:
        return
    try:
        from trn_agent_boot.trn_boot import _ntff_profile_via_ctypes

        hook = _ntff_profile_via_ctypes("/opt/axon/libaxon_pjrt.so")
    except Exception:
        hook = None
    mod = types.ModuleType("antenv.axon_hooks")
    mod._hook = hook
    mod.get_axon_ntff_profile_hook = lambda: mod._hook
    mod.set_axon_ntff_profile_hook = lambda h: setattr(mod, "_hook", h)
    sys.modules["antenv.axon_hooks"] = mod
    try:
        import antenv

        antenv.axon_hooks = mod
    except Exception:
        pass


_install_ntff_hook()

import concourse.bass as bass  # noqa: E402
import concourse.mybir as mybir  # noqa: E402
import concourse.tile as tile  # noqa: E402
from concourse import bacc  # noqa: E402
from concourse.bass import ds  # noqa: E402
from concourse.bass_utils import run_bass_kernel_spmd  # noqa: E402

B, S, D, H = 2, 2048, 1024, 16
R = B * S              # 4096 flattened rows
NC = 8                 # cores
HPC = H // NC          # 2 heads per core
CW = D // NC           # 128 ctx cols per core
DH = D // H            # 64 head dim
RW = R // NC           # 512 output rows per core
F32 = mybir.dt.float32
F16 = mybir.dt.float16
I16 = mybir.dt.int16
FP = np.float32

# Schraudolph fp16 exp constants: i16 = round(raw_score * A + B);
# bitcast(i16) ~= exp(raw_score / 8).
A_SCH = float(1024.0 * np.log2(np.e) / 8.0)
B_SCH = float(15.0 * 1024.0 - 0.043677448 * 1024.0)

# exp engine per kt (16 key tiles per q tile): a=Scalar(exact LUT),
# v=Vector(Schraudolph).  GpSimd cannot read PSUM, so it gets no slots.
EXP_ENG = "avavavavavavavaa"


def _perm_local() -> np.ndarray:
    """Within a core's 128-col slice: [A-evens, A-odds, B-evens, B-odds]."""
    a_ev = np.arange(0, 64, 2)
    a_od = np.arange(1, 64, 2)
    return np.concatenate([a_ev, a_od, 64 + a_ev, 64 + a_od])


def _build_program():
    nc = bacc.Bacc("TRN2", target_bir_lowering=False, debug=False, num_devices=NC)

    # ---- external I/O ----
    xq_d = nc.dram_tensor("xq", [D, R], F16, kind="ExternalInput").ap()
    xk_d = nc.dram_tensor("xk", [D, R], F16, kind="ExternalInput").ap()
    xv_d = nc.dram_tensor("xv", [D, R], F16, kind="ExternalInput").ap()
    wq_d = nc.dram_tensor("wq", [D, CW], F16, kind="ExternalInput").ap()
    wk_d = nc.dram_tensor("wk", [D, CW], F16, kind="ExternalInput").ap()
    wv_d = nc.dram_tensor("wv", [D, CW], F16, kind="ExternalInput").ap()
    wp_d = nc.dram_tensor("wp", [D, D], F16, kind="ExternalInput").ap()
    cs_d = nc.dram_tensor("cs", [128, R], F16, kind="ExternalInput").ap()
    sn_d = nc.dram_tensor("sn", [128, R], F16, kind="ExternalInput").ap()
    sw_d = nc.dram_tensor("sw", [128, 128], F16, kind="ExternalInput").ap()
    id_d = nc.dram_tensor("ident", [128, 128], F16, kind="ExternalInput").ap()
    on_d = nc.dram_tensor("ones", [128, 64], F16, kind="ExternalInput").ap()
    bi_d = nc.dram_tensor("bias", [128, D], F32, kind="ExternalInput").ap()
    out_d = nc.dram_tensor("out", [RW, D], F32, kind="ExternalOutput").ap()

    KT = D // 128   # 8 contraction tiles for projections
    RT = R // 512   # 8 row tiles
    QT = S // 512   # 4 q tiles per batch
    ST = S // 128   # 16 k tiles per batch

    with tile.TileContext(nc) as tc:
        with (
            tc.tile_pool(name="const", bufs=1) as const,
            tc.tile_pool(name="persist", bufs=1) as persist,
            tc.tile_pool(name="dram", bufs=1, space="DRAM") as dram,
            tc.tile_pool(name="wqkv", bufs=1) as wpool,
            tc.tile_pool(name="trig", bufs=1) as trig,
            tc.tile_pool(name="xin", bufs=3) as xin,
            tc.tile_pool(name="vtmp", bufs=1) as vtmp,
            tc.tile_pool(name="rope", bufs=3) as rp,
            tc.tile_pool(name="epool", bufs=1) as ep,
            tc.tile_pool(name="norm", bufs=1) as npl,
            tc.tile_pool(name="oio", bufs=2) as oio,
            tc.tile_pool(name="pp", bufs=1, space="PSUM") as pp,
            tc.tile_pool(name="spsum", bufs=1, space="PSUM") as sp,
            tc.tile_pool(name="cpsum", bufs=1, space="PSUM") as cp,
        ):
            # ---- constants / weights ----
            # Only wk + sw are DMA'd up front; the rest are issued at their
            # first use point so the opening k-proj isn't DMA-starved.
            wk_sb = wpool.tile([128, KT, CW], F16)
            nc.sync.dma_start(wk_sb[:], wk_d.rearrange("(kt p) m -> p kt m", p=128))
            sw_sb = const.tile([128, 128], F16)
            nc.sync.dma_start(sw_sb[:], sw_d[:])
            id_sb = const.tile([128, 128], F16)
            on_sb = const.tile([128, 64], F16)
            wv_sb = wpool.tile([128, KT, CW], F16)
            wq_sb = wpool.tile([128, KT, CW], F16)
            cs_sb = trig.tile([128, R], F16)
            sn_sb = trig.tile([128, R], F16)
            bi_sb = const.tile([128, D], F32)
            wp_sb = const.tile([128, KT, D], F16)

            qpT = persist.tile([128, R], F16)
            kpT = persist.tile([128, R], F16)
            vaug = persist.tile([128, R // 128, 256], F16)
            vpT = vtmp.tile([128, R], F16)

            def emit_vaug_init():
                nc.vector.tensor_copy(
                    vaug[:, :, 64:128],
                    on_sb[:, :, None].rearrange("p o n -> p n o").to_broadcast((128, R // 128, 64)),
                )
                nc.vector.tensor_copy(
                    vaug[:, :, 192:256],
                    on_sb[:, :, None].rearrange("p o n -> p n o").to_broadcast((128, R // 128, 64)),
                )

            _chunk_rows = [128, 128, 128, 64, 64]
            a2a_ins = [dram.tile([NC, 128, _chunk_rows[p]], F16,
                                 name=f"a2ain{p}", tag=f"a2ain{p}")
                       for p in range(5)]
            a2a_outs = [dram.tile([NC, 128, _chunk_rows[p]], F16,
                                  name=f"a2aout{p}", tag=f"a2aout{p}")
                        for p in range(5)]

            RTB = RT // B     # 4 row-tiles per batch
            CHUNKS = [(0, 2), (2, 4), (4, 6), (6, 7), (7, 8)]  # groups per A2A

            def prefetch_x(ti, rt):
                xd = (xk_d, xv_d, xq_d)[ti]
                rsl = ds(rt * 512, 512)
                x_sb = xin.tile([128, KT, 512], F16, tag="xin", name="x_sb")
                # split so the first contraction tiles land (and unblock the
                # matmuls) before the whole slice has arrived
                xr = xd.rearrange("(kt p) r -> p kt r", p=128)
                nc.sync.dma_start(x_sb[:, 0:2], xr[:, 0:2, rsl])
                nc.sync.dma_start(x_sb[:, 2:KT], xr[:, 2:KT, rsl])
                return x_sb

            def emit_proj_tile(ti, rt, x_sb=None):
                xd, wsb, dest, do_rope = (
                    (xk_d, wk_sb, kpT, True),
                    (xv_d, wv_sb, vpT, True),
                    (xq_d, wq_sb, qpT, False),
                )[ti]
                rsl = ds(rt * 512, 512)
                if x_sb is None:
                    x_sb = prefetch_x(ti, rt)
                ps = pp.tile([128, 512], F32, tag="proj", bufs=2, name="ps")
                for kt in range(KT):
                    nc.tensor.matmul(
                        ps[:], wsb[:, kt], x_sb[:, kt],
                        start=(kt == 0), stop=(kt == KT - 1),
                    )
                if not do_rope:
                    nc.vector.tensor_copy(dest[:, rsl], ps[:])
                else:
                    raw = rp.tile([128, 512], F16, tag="raw", name="raw")
                    nc.vector.tensor_copy(raw[:], ps[:])
                    sps = pp.tile([128, 512], F32, tag="proj", bufs=2, name="sps")
                    nc.tensor.matmul(sps[:], sw_sb[:], raw[:], start=True, stop=True)
                    t1 = rp.tile([128, 512], F32, tag="t1", name="t1")
                    nc.gpsimd.tensor_tensor(
                        t1[:], raw[:], cs_sb[:, rsl], mybir.AluOpType.mult)
                    t2 = rp.tile([128, 512], F32, tag="t2", name="t2")
                    nc.vector.tensor_tensor(
                        t2[:], sps[:], sn_sb[:, rsl], mybir.AluOpType.mult)
                    nc.vector.tensor_tensor(
                        dest[:, rsl], t1[:], t2[:], mybir.AluOpType.add)

            def emit_vtrans(ct):
                tpx = pp.tile([128, 512], F16, tag="proj", bufs=2, name="tpx")
                tp = tpx[:, 0:128]
                nc.tensor.transpose(tp, vpT[:, ds(ct * 128, 128)], id_sb[:])
                nc.vector.tensor_copy(vaug[:, ct, 0:64], tp[:, 0:64])
                nc.vector.tensor_copy(vaug[:, ct, 128:192], tp[:, 64:128])

            def emit_attn_qt(bb, qt):
                """Returns a deferred-outproj closure (or None): caller emits
                it after queueing more PE work, so the output-projection
                matmuls (which wait on the AllToAll) don't head-of-line-block
                the PE FIFO."""
                qsl = ds(bb * S + qt * 512, 512)
                # E[:, kt, 0, :] = head A weights, [:, kt, 1, :] = head B
                E = ep.tile([128, ST, 2, 512], F16, tag="E", bufs=2, name="E")
                cA = cp.tile([128, 512], F32, tag="cA", bufs=1, name="cA")
                cB = cp.tile([128, 512], F32, tag="cB", bufs=1, name="cB")

                def ctx_kt(kt):
                    ct = bb * ST + kt
                    nc.tensor.matmul(cA[:], vaug[:, ct, 0:128], E[:, kt, 0],
                                     start=(kt == 0), stop=(kt == ST - 1))
                    nc.tensor.matmul(cB[:], vaug[:, ct, 128:256], E[:, kt, 1],
                                     start=(kt == 0), stop=(kt == ST - 1))

                for kt in range(ST):
                    ksl = ds(bb * S + kt * 128, 128)
                    ps = sp.tile([128, 2, 512], F32, tag="sc", bufs=2,
                                 name="ps_sc")
                    nc.tensor.matmul(
                        ps[:, 0], kpT[0:64, ksl], qpT[0:64, qsl],
                        start=True, stop=True, tile_position=(0, 0))
                    nc.tensor.matmul(
                        ps[:, 1], kpT[64:128, ksl], qpT[64:128, qsl],
                        start=True, stop=True, tile_position=(64, 0))
                    eng = EXP_ENG[kt]
                    if eng == "a":
                        nc.scalar.activation(
                            E[:, kt], ps[:],
                            mybir.ActivationFunctionType.Exp, scale=0.125)
                    elif eng == "v":
                        nc.vector.tensor_scalar(
                            E[:, kt].bitcast(I16), ps[:], A_SCH, B_SCH,
                            mybir.AluOpType.mult, mybir.AluOpType.add)
                    else:
                        nc.gpsimd.tensor_scalar(
                            E[:, kt].bitcast(I16), ps[:], A_SCH, B_SCH,
                            mybir.AluOpType.mult, mybir.AluOpType.add)
                    if kt >= 2:
                        ctx_kt(kt - 2)
                ctx_kt(ST - 2)
                ctx_kt(ST - 1)

                zhi = npl.tile([128, 1024], F32, tag="zhi", name="zhi")
                nc.vector.tensor_copy(zhi[64:128, 0:512], cA[64:128])
                nc.vector.tensor_copy(zhi[64:128, 512:1024], cB[64:128])
                zlo = npl.tile([64, 1024], F32, tag="zlo", name="zlo")
                nc.sync.dma_start(zlo[:], zhi[64:128, :])
                zr = npl.tile([64, 1024], F32, tag="zr", name="zr")
                nc.vector.reciprocal_approx_fast(zr[:], zlo[:])
                ctxA = npl.tile([64, 512], F16, tag="ctxA", name="ctxA")
                nc.vector.tensor_tensor(
                    ctxA[:], cA[0:64], zr[:, 0:512], mybir.AluOpType.mult)
                ctxB = npl.tile([64, 512], F16, tag="ctxB", name="ctxB")
                nc.vector.tensor_tensor(
                    ctxB[:], cB[0:64], zr[:, 512:1024], mybir.AluOpType.mult)
                shard = bb * QT + qt
                ck = next(i for i, (a, b) in enumerate(CHUNKS) if a <= shard < b)
                a, b = CHUNKS[ck]
                rsl2 = ds((shard - a) * 64, 64)
                nc.sync.dma_start(
                    a2a_ins[ck][:, 0:64, rsl2].rearrange("j p r -> p j r"),
                    ctxA[:].rearrange("p (j r) -> p j r", j=NC))
                nc.sync.dma_start(
                    a2a_ins[ck][:, 64:128, rsl2].rearrange("j p r -> p j r"),
                    ctxB[:].rearrange("p (j r) -> p j r", j=NC))
                if shard != b - 1:
                    return None
                nc.gpsimd.collective_compute(
                    "AllToAll",
                    mybir.AluOpType.bypass,
                    replica_groups=[list(range(NC))],
                    ins=[a2a_ins[ck].opt()],
                    outs=[a2a_outs[ck].opt()],
                )
                nrows = (b - a) * 64
                lh = oio.tile([128, NC, 128], F16, tag="lh", name="lh")
                # gpsimd queue (not sync): this DMA waits on the collective,
                # and on the sync queue it would FIFO-block later x prefetches
                nc.gpsimd.dma_start(
                    lh[:, :, 0:nrows],
                    a2a_outs[ck][:].rearrange("j p r -> p j r"),
                )

                def do_outproj():
                    for oc in range(2):
                        po = pp.tile([128, 512], F32, tag="proj", bufs=2,
                                     name="po")
                        for j in range(NC):
                            nc.tensor.matmul(
                                po[0:nrows], lh[:, j, 0:nrows],
                                wp_sb[:, j, ds(oc * 512, 512)],
                                start=(j == 0), stop=(j == NC - 1))
                        ob = oio.tile([128, 512], F32, tag="ob", name="ob")
                        nc.vector.tensor_tensor(
                            ob[0:nrows], po[0:nrows], bi_sb[0:nrows, ds(oc * 512, 512)],
                            mybir.AluOpType.add)
                        nc.gpsimd.dma_start(
                            out_d[ds(a * 64, nrows), ds(oc * 512, 512)], ob[0:nrows])

                return do_outproj

            # ---- batch 0 projections ----
            # DMA order matters at startup: the first k-proj tile needs only
            # wk + x(0,0) (+ trig/sw for its rope) — everything else later.
            x00 = prefetch_x(0, 0)
            x01 = prefetch_x(0, 1)
            nc.sync.dma_start(cs_sb[:], cs_d[:])
            nc.sync.dma_start(sn_sb[:], sn_d[:])
            emit_proj_tile(0, 0, x00)
            emit_proj_tile(0, 1, x01)
            nc.sync.dma_start(wv_sb[:], wv_d.rearrange("(kt p) m -> p kt m", p=128))
            nc.sync.dma_start(wq_sb[:], wq_d.rearrange("(kt p) m -> p kt m", p=128))
            nc.sync.dma_start(id_sb[:], id_d[:])
            nc.sync.dma_start(on_sb[:], on_d[:])
            for rt in range(2, RTB):
                emit_proj_tile(0, rt)
            emit_vaug_init()
            for ti in (1, 2):
                for rt in range(0, RTB):
                    emit_proj_tile(ti, rt)
            nc.sync.dma_start(bi_sb[:], bi_d[:])
            nc.sync.dma_start(wp_sb[:], wp_d.rearrange("(kt p) o -> p kt o", p=128))
            for ct in range(0, S // 128):
                emit_vtrans(ct)
            # ---- batch 0 attention; batch 1 projections interleaved ----
            # piece order: k then v (so b1 vtrans can run right after qt3),
            # then q (only needed once b1 attention starts)
            b1_pieces = ([(0, rt) for rt in range(RTB, 2 * RTB)]
                         + [(1, rt) for rt in range(RTB, 2 * RTB)]
                         + [(2, rt) for rt in range(RTB, 2 * RTB)])
            pending = None
            for qt in range(QT):
                # prefetch piece inputs BEFORE the attention emission so these
                # DMAs aren't queued behind the chunk's collective-dependent
                # traffic
                pieces = b1_pieces[qt * 3:(qt + 1) * 3]
                xs = [prefetch_x(ti, rt) for ti, rt in pieces]
                nxt = emit_attn_qt(0, qt)
                for (ti, rt), x_sb in zip(pieces, xs):
                    emit_proj_tile(ti, rt, x_sb)
                if pending is not None:
                    pending()
                pending = nxt
            # b1 vtrans burst overlaps the chunk-1 collective latency
            for ct in range(S // 128, 2 * (S // 128)):
                emit_vtrans(ct)
            if pending is not None:
                pending()
                pending = None
            for qt in range(QT):
                nxt = emit_attn_qt(1, qt)
                if pending is not None:
                    pending()
                pending = nxt
            if pending is not None:
                pending()

    nc.compile()
    return nc


_PROGRAM = None


def _get_program():
    global _PROGRAM
    if _PROGRAM is None:
        _PROGRAM = _build_program()
    return _PROGRAM


def _host_prep(q, k, v, Wq, Wk, Wv, Wp, bp):
    """Build the 8 per-core input maps."""
    rr = lambda a: np.ascontiguousarray(a, dtype=np.float32).astype(np.float16)
    xqT = rr(q.reshape(R, D).T)
    xkT = rr(k.reshape(R, D).T)
    xvT = rr(v.reshape(R, D).T)

    pl = _perm_local()
    perm_global = np.concatenate([128 * c + pl for c in range(NC)])
    wpT = rr(np.ascontiguousarray(Wp.T[perm_global, :]))

    # trig tables
    half = D // 2
    pos = np.arange(S, dtype=np.float64)
    theta = 1.0 / (10000.0 ** (2.0 * np.arange(half, dtype=np.float64) / D))
    ang = pos[:, None] * theta[None, :]          # [S, half]
    cosf = np.cos(ang).astype(FP)                # [S, half]
    sinf = np.sin(ang).astype(FP)

    sw = np.zeros((128, 128), np.float16)
    for m in range(128):
        p = (m + 32) % 64 + 64 * (m // 64)
        sw[p, m] = 1.0
    ident = np.eye(128, dtype=np.float16)
    ones = np.ones((128, 64), np.float16)
    bias = np.broadcast_to(bp.astype(FP), (128, D)).copy()

    in_maps = []
    for c in range(NC):
        cols = 128 * c + pl
        wq_c = rr(np.ascontiguousarray(Wq[cols, :].T))
        wk_c = rr(np.ascontiguousarray(Wk[cols, :].T))
        wv_c = rr(np.ascontiguousarray(Wv[cols, :].T))
        # pair index per partition p (see _perm_local ordering)
        j = np.empty(128, np.int64)
        j[0:32] = 64 * c + np.arange(32)
        j[32:64] = 64 * c + np.arange(32)
        j[64:96] = 64 * c + 32 + np.arange(32)
        j[96:128] = 64 * c + 32 + np.arange(32)
        cs1 = cosf[:, j].T                        # [128, S]
        sn1 = sinf[:, j].T.copy()
        sn1[0:32] *= -1.0
        sn1[64:96] *= -1.0
        cs = np.tile(cs1, (1, B)).astype(np.float16)      # [128, R]
        sn = np.tile(sn1, (1, B)).astype(np.float16)
        in_maps.append({
            "xq": xqT, "xk": xkT, "xv": xvT,
            "wq": wq_c, "wk": wk_c, "wv": wv_c,
            "wp": wpT, "cs": cs, "sn": sn,
            "sw": sw, "ident": ident, "ones": ones, "bias": bias,
        })
    return in_maps


def run(inputs, trace=False, trace_cores=None):
    nc = _get_program()
    in_maps = _host_prep(**inputs)
    res = run_bass_kernel_spmd(
        nc, in_maps, core_ids=list(range(NC)), trace=trace,
        trace_cores=trace_cores,
    )
    outs = np.stack([res.results[c]["out"] for c in range(NC)])  # [c, 512, D]
    # local row (128p + 64g' + i) on core c == global row 512*(2p+g') + 64c + i
    lo = outs.reshape(NC, NC, 64, D)              # [core, (2p,g'), i, D]
    full = lo.transpose(1, 0, 2, 3).reshape(B, S, D)
    return full, res


def kernel(**inputs) -> np.ndarray:
    trace = bool(int(os.environ.get("TRN_TRACE", "0")))
    full, res = run(inputs, trace=trace)
    if trace and res.exec_time_ns is not None:
        print(f"HW exec time: {res.exec_time_ns} ns")
    return full


# revision 17
# speedup vs baseline: 1.2471x; 1.0161x over previous
"""Multi-head attention (RoPE on k/v) Bass kernel for 8 TRN2 NeuronCores.

Sharding: tensor-parallel over heads (2 heads/core, both batches) for the
QKV projections + attention; one AllToAll redistributes context to a
row-sharded output projection. All matmuls fp16.

v2: scores matmul pairs run concurrently in disjoint PE row-groups
(tile_position (0,0)/(64,0)); softmax exp is split across the Scalar
engine (exact LUT exp) and Vector/GpSimd (one-instruction Schraudolph
exp: round(A*s+B) -> int16, bit-identical to an fp16 exponential
approximation, max rel err ~2%).

Self-contained: hardcodes shapes from the problem spec.
"""
import os
import sys
import types

import numpy as np


def _install_ntff_hook():
    """antenv.axon_hooks is missing from this image; synthesize it so
    run_bass_kernel_spmd(trace=True) works. Harmless when trace=False."""
    if "antenv.axon_hooks" in sys.modules:
        return
    try:
        from trn_agent_boot.trn_boot import _ntff_profile_via_ctypes

        hook = _ntff_profile_via_ctypes("/opt/axon/libaxon_pjrt.so")
    except Exception:
        hook = None
    mod = types.ModuleType("antenv.axon_hooks")
    mod._hook = hook
    mod.get_axon_ntff_profile_hook = lambda: mod._hook
    mod.set_axon_ntff_profile_hook = lambda h: setattr(mod, "_hook", h)
    sys.modules["antenv.axon_hooks"] = mod
    try:
        import antenv

        antenv.axon_hooks = mod
    except Exception:
        pass


_install_ntff_hook()

import concourse.bass as bass  # noqa: E402
import concourse.mybir as mybir  # noqa: E402
import concourse.tile as tile  # noqa: E402
from concourse import bacc  # noqa: E402
from concourse.bass import ds  # noqa: E402
from concourse.bass_utils import run_bass_kernel_spmd  # noqa: E402

B, S, D, H = 2, 2048, 1024, 16
R = B * S              # 4096 flattened rows
NC = 8                 # cores
HPC = H // NC          # 2 heads per core
CW = D // NC           # 128 ctx cols per core
DH = D // H            # 64 head dim
RW = R // NC           # 512 output rows per core
F32 = mybir.dt.float32
F16 = mybir.dt.float16
I16 = mybir.dt.int16
FP = np.float32

# Schraudolph fp16 exp constants: i16 = round(raw_score * A + B);
# bitcast(i16) ~= exp(raw_score / 8).
A_SCH = float(1024.0 * np.log2(np.e) / 8.0)
B_SCH = float(15.0 * 1024.0 - 0.043677448 * 1024.0)

# exp engine per kt (16 key tiles per q tile): a=Scalar(exact LUT),
# v=Vector(Schraudolph).  GpSimd cannot read PSUM, so it gets no slots.
EXP_ENG = "aavavaavavaavava"


def _perm_local() -> np.ndarray:
    """Within a core's 128-col slice: [A-evens, A-odds, B-evens, B-odds]."""
    a_ev = np.arange(0, 64, 2)
    a_od = np.arange(1, 64, 2)
    return np.concatenate([a_ev, a_od, 64 + a_ev, 64 + a_od])


def _build_program():
    nc = bacc.Bacc("TRN2", target_bir_lowering=False, debug=False, num_devices=NC)

    # ---- external I/O ----
    xq_d = nc.dram_tensor("xq", [D, R], F16, kind="ExternalInput").ap()
    xk_d = nc.dram_tensor("xk", [D, R], F16, kind="ExternalInput").ap()
    xv_d = nc.dram_tensor("xv", [D, R], F16, kind="ExternalInput").ap()
    wq_d = nc.dram_tensor("wq", [D, CW], F16, kind="ExternalInput").ap()
    wk_d = nc.dram_tensor("wk", [D, CW], F16, kind="ExternalInput").ap()
    wv_d = nc.dram_tensor("wv", [D, CW], F16, kind="ExternalInput").ap()
    wp_d = nc.dram_tensor("wp", [D, D], F16, kind="ExternalInput").ap()
    cs_d = nc.dram_tensor("cs", [128, R], F16, kind="ExternalInput").ap()
    sn_d = nc.dram_tensor("sn", [128, R], F16, kind="ExternalInput").ap()
    sw_d = nc.dram_tensor("sw", [128, 128], F16, kind="ExternalInput").ap()
    id_d = nc.dram_tensor("ident", [128, 128], F16, kind="ExternalInput").ap()
    on_d = nc.dram_tensor("ones", [128, 64], F16, kind="ExternalInput").ap()
    bi_d = nc.dram_tensor("bias", [128, D], F32, kind="ExternalInput").ap()
    out_d = nc.dram_tensor("out", [RW, D], F32, kind="ExternalOutput").ap()

    KT = D // 128   # 8 contraction tiles for projections
    RT = R // 512   # 8 row tiles
    QT = S // 512   # 4 q tiles per batch
    ST = S // 128   # 16 k tiles per batch

    with tile.TileContext(nc) as tc:
        with (
            tc.tile_pool(name="const", bufs=1) as const,
            tc.tile_pool(name="persist", bufs=1) as persist,
            tc.tile_pool(name="dram", bufs=1, space="DRAM") as dram,
            tc.tile_pool(name="wqkv", bufs=1) as wpool,
            tc.tile_pool(name="trig", bufs=1) as trig,
            tc.tile_pool(name="xin", bufs=3) as xin,
            tc.tile_pool(name="vtmp", bufs=1) as vtmp,
            tc.tile_pool(name="rope", bufs=3) as rp,
            tc.tile_pool(name="epool", bufs=1) as ep,
            tc.tile_pool(name="norm", bufs=1) as npl,
            tc.tile_pool(name="oio", bufs=2) as oio,
            tc.tile_pool(name="pp", bufs=1, space="PSUM") as pp,
            tc.tile_pool(name="spsum", bufs=1, space="PSUM") as sp,
            tc.tile_pool(name="cpsum", bufs=1, space="PSUM") as cp,
        ):
            # ---- constants / weights ----
            # Only wk + sw are DMA'd up front; the rest are issued at their
            # first use point so the opening k-proj isn't DMA-starved.
            wk_sb = wpool.tile([128, KT, CW], F16)
            nc.sync.dma_start(wk_sb[:], wk_d.rearrange("(kt p) m -> p kt m", p=128))
            sw_sb = const.tile([128, 128], F16)
            nc.sync.dma_start(sw_sb[:], sw_d[:])
            id_sb = const.tile([128, 128], F16)
            on_sb = const.tile([128, 64], F16)
            wv_sb = wpool.tile([128, KT, CW], F16)
            wq_sb = wpool.tile([128, KT, CW], F16)
            cs_sb = trig.tile([128, R], F16)
            sn_sb = trig.tile([128, R], F16)
            bi_sb = const.tile([128, D], F32)
            wp_sb = const.tile([128, KT, D], F16)

            qpT = persist.tile([128, R], F16)
            kpT = persist.tile([128, R], F16)
            vaug = persist.tile([128, R // 128, 256], F16)
            vpT = vtmp.tile([128, R], F16)

            def emit_vaug_init():
                nc.vector.tensor_copy(
                    vaug[:, :, 64:128],
                    on_sb[:, :, None].rearrange("p o n -> p n o").to_broadcast((128, R // 128, 64)),
                )
                nc.vector.tensor_copy(
                    vaug[:, :, 192:256],
                    on_sb[:, :, None].rearrange("p o n -> p n o").to_broadcast((128, R // 128, 64)),
                )

            _chunk_rows = [128, 128, 128, 64, 64]
            a2a_ins = [dram.tile([NC, 128, _chunk_rows[p]], F16,
                                 name=f"a2ain{p}", tag=f"a2ain{p}")
                       for p in range(5)]
            a2a_outs = [dram.tile([NC, 128, _chunk_rows[p]], F16,
                                  name=f"a2aout{p}", tag=f"a2aout{p}")
                        for p in range(5)]

            RTB = RT // B     # 4 row-tiles per batch
            CHUNKS = [(0, 2), (2, 4), (4, 6), (6, 7), (7, 8)]  # groups per A2A

            def prefetch_x(ti, rt):
                xd = (xk_d, xv_d, xq_d)[ti]
                rsl = ds(rt * 512, 512)
                x_sb = xin.tile([128, KT, 512], F16, tag="xin", name="x_sb")
                # split so the first contraction tiles land (and unblock the
                # matmuls) before the whole slice has arrived
                xr = xd.rearrange("(kt p) r -> p kt r", p=128)
                nc.sync.dma_start(x_sb[:, 0:2], xr[:, 0:2, rsl])
                nc.sync.dma_start(x_sb[:, 2:KT], xr[:, 2:KT, rsl])
                return x_sb

            def emit_proj_tile(ti, rt, x_sb=None):
                xd, wsb, dest, do_rope = (
                    (xk_d, wk_sb, kpT, True),
                    (xv_d, wv_sb, vpT, True),
                    (xq_d, wq_sb, qpT, False),
                )[ti]
                rsl = ds(rt * 512, 512)
                if x_sb is None:
                    x_sb = prefetch_x(ti, rt)
                ps = pp.tile([128, 512], F32, tag="proj", bufs=2, name="ps")
                for kt in range(KT):
                    nc.tensor.matmul(
                        ps[:], wsb[:, kt], x_sb[:, kt],
                        start=(kt == 0), stop=(kt == KT - 1),
                    )
                if not do_rope:
                    nc.vector.tensor_copy(dest[:, rsl], ps[:])
                else:
                    raw = rp.tile([128, 512], F16, tag="raw", name="raw")
                    nc.scalar.activation(raw[:], ps[:],
                                         mybir.ActivationFunctionType.Copy)
                    sps = pp.tile([128, 512], F32, tag="proj", bufs=2, name="sps")
                    nc.tensor.matmul(sps[:], sw_sb[:], raw[:], start=True, stop=True)
                    t1 = rp.tile([128, 512], F32, tag="t1", name="t1")
                    nc.gpsimd.tensor_tensor(
                        t1[:], raw[:], cs_sb[:, rsl], mybir.AluOpType.mult)
                    t2 = rp.tile([128, 512], F32, tag="t2", name="t2")
                    nc.vector.tensor_tensor(
                        t2[:], sps[:], sn_sb[:, rsl], mybir.AluOpType.mult)
                    nc.vector.tensor_tensor(
                        dest[:, rsl], t1[:], t2[:], mybir.AluOpType.add)

            def emit_vtrans(ct):
                tpx = pp.tile([128, 512], F16, tag="proj", bufs=2, name="tpx")
                tp = tpx[:, 0:128]
                nc.tensor.transpose(tp, vpT[:, ds(ct * 128, 128)], id_sb[:])
                nc.vector.tensor_copy(vaug[:, ct, 0:64], tp[:, 0:64])
                nc.vector.tensor_copy(vaug[:, ct, 128:192], tp[:, 64:128])

            def emit_attn_qt(bb, qt):
                """Returns a deferred-outproj closure (or None): caller emits
                it after queueing more PE work, so the output-projection
                matmuls (which wait on the AllToAll) don't head-of-line-block
                the PE FIFO."""
                qsl = ds(bb * S + qt * 512, 512)
                # E[:, kt, 0, :] = head A weights, [:, kt, 1, :] = head B
                E = ep.tile([128, ST, 2, 512], F16, tag="E", bufs=2, name="E")
                cA = cp.tile([128, 512], F32, tag="cA", bufs=1, name="cA")
                cB = cp.tile([128, 512], F32, tag="cB", bufs=1, name="cB")

                def ctx_kt(kt):
                    ct = bb * ST + kt
                    nc.tensor.matmul(cA[:], vaug[:, ct, 0:128], E[:, kt, 0],
                                     start=(kt == 0), stop=(kt == ST - 1))
                    nc.tensor.matmul(cB[:], vaug[:, ct, 128:256], E[:, kt, 1],
                                     start=(kt == 0), stop=(kt == ST - 1))

                for kt in range(ST):
                    ksl = ds(bb * S + kt * 128, 128)
                    ps = sp.tile([128, 2, 512], F32, tag="sc", bufs=2,
                                 name="ps_sc")
                    nc.tensor.matmul(
                        ps[:, 0], kpT[0:64, ksl], qpT[0:64, qsl],
                        start=True, stop=True, tile_position=(0, 0))
                    nc.tensor.matmul(
                        ps[:, 1], kpT[64:128, ksl], qpT[64:128, qsl],
                        start=True, stop=True, tile_position=(64, 0))
                    eng = EXP_ENG[kt]
                    if eng == "a":
                        nc.scalar.activation(
                            E[:, kt], ps[:],
                            mybir.ActivationFunctionType.Exp, scale=0.125)
                    elif eng == "v":
                        nc.vector.tensor_scalar(
                            E[:, kt].bitcast(I16), ps[:], A_SCH, B_SCH,
                            mybir.AluOpType.mult, mybir.AluOpType.add)
                    else:
                        nc.gpsimd.tensor_scalar(
                            E[:, kt].bitcast(I16), ps[:], A_SCH, B_SCH,
                            mybir.AluOpType.mult, mybir.AluOpType.add)
                    if kt >= 2:
                        ctx_kt(kt - 2)
                ctx_kt(ST - 2)
                ctx_kt(ST - 1)

                zhi = npl.tile([128, 1024], F32, tag="zhi", name="zhi")
                nc.vector.tensor_copy(zhi[64:128, 0:512], cA[64:128])
                nc.vector.tensor_copy(zhi[64:128, 512:1024], cB[64:128])
                zlo = npl.tile([64, 1024], F32, tag="zlo", name="zlo")
                nc.sync.dma_start(zlo[:], zhi[64:128, :])
                zr = npl.tile([64, 1024], F32, tag="zr", name="zr")
                nc.vector.reciprocal_approx_fast(zr[:], zlo[:])
                ctxA = npl.tile([64, 512], F16, tag="ctxA", name="ctxA")
                nc.vector.tensor_tensor(
                    ctxA[:], cA[0:64], zr[:, 0:512], mybir.AluOpType.mult)
                ctxB = npl.tile([64, 512], F16, tag="ctxB", name="ctxB")
                nc.vector.tensor_tensor(
                    ctxB[:], cB[0:64], zr[:, 512:1024], mybir.AluOpType.mult)
                shard = bb * QT + qt
                ck = next(i for i, (a, b) in enumerate(CHUNKS) if a <= shard < b)
                a, b = CHUNKS[ck]
                rsl2 = ds((shard - a) * 64, 64)
                nc.sync.dma_start(
                    a2a_ins[ck][:, 0:64, rsl2].rearrange("j p r -> p j r"),
                    ctxA[:].rearrange("p (j r) -> p j r", j=NC))
                nc.sync.dma_start(
                    a2a_ins[ck][:, 64:128, rsl2].rearrange("j p r -> p j r"),
                    ctxB[:].rearrange("p (j r) -> p j r", j=NC))
                if shard != b - 1:
                    return None, None
                nrows = (b - a) * 64

                def do_collective():
                    nc.gpsimd.collective_compute(
                        "AllToAll",
                        mybir.AluOpType.bypass,
                        replica_groups=[list(range(NC))],
                        ins=[a2a_ins[ck].opt()],
                        outs=[a2a_outs[ck].opt()],
                    )

                def do_outproj():
                    lh = oio.tile([128, NC, 128], F16, tag="lh", name="lh")
                    # gpsimd queue (not sync): this DMA waits on the
                    # collective; on the sync queue it would FIFO-block later
                    # x prefetches / staging
                    nc.gpsimd.dma_start(
                        lh[:, :, 0:nrows],
                        a2a_outs[ck][:].rearrange("j p r -> p j r"),
                    )
                    for oc in range(2):
                        po = pp.tile([128, 512], F32, tag="proj", bufs=2,
                                     name="po")
                        for j in range(NC):
                            nc.tensor.matmul(
                                po[0:nrows], lh[:, j, 0:nrows],
                                wp_sb[:, j, ds(oc * 512, 512)],
                                start=(j == 0), stop=(j == NC - 1))
                        ob = oio.tile([128, 512], F32, tag="ob", name="ob")
                        nc.vector.tensor_tensor(
                            ob[0:nrows], po[0:nrows], bi_sb[0:nrows, ds(oc * 512, 512)],
                            mybir.AluOpType.add)
                        nc.gpsimd.dma_start(
                            out_d[ds(a * 64, nrows), ds(oc * 512, 512)], ob[0:nrows])

                return do_collective, do_outproj

            # ---- batch 0 projections ----
            # DMA order matters at startup: the first k-proj tile needs only
            # wk + x(0,0) (+ trig/sw for its rope) — everything else later.
            x00 = prefetch_x(0, 0)
            x01 = prefetch_x(0, 1)
            nc.sync.dma_start(cs_sb[:], cs_d[:])
            nc.sync.dma_start(sn_sb[:], sn_d[:])
            emit_proj_tile(0, 0, x00)
            emit_proj_tile(0, 1, x01)
            nc.sync.dma_start(wv_sb[:], wv_d.rearrange("(kt p) m -> p kt m", p=128))
            nc.sync.dma_start(wq_sb[:], wq_d.rearrange("(kt p) m -> p kt m", p=128))
            nc.sync.dma_start(id_sb[:], id_d[:])
            nc.sync.dma_start(on_sb[:], on_d[:])
            for rt in range(2, RTB):
                emit_proj_tile(0, rt)
            emit_vaug_init()
            for ti in (1, 2):
                for rt in range(0, RTB):
                    emit_proj_tile(ti, rt)
            nc.sync.dma_start(bi_sb[:], bi_d[:])
            nc.sync.dma_start(wp_sb[:], wp_d.rearrange("(kt p) o -> p kt o", p=128))
            for ct in range(0, S // 128):
                emit_vtrans(ct)
            # ---- batch 0 attention; batch 1 projections interleaved ----
            # piece order: k then v (so b1 vtrans can run right after qt3),
            # then q (only needed once b1 attention starts)
            b1_pieces = ([(0, rt) for rt in range(RTB, 2 * RTB)]
                         + [(1, rt) for rt in range(RTB, 2 * RTB)]
                         + [(2, rt) for rt in range(RTB, 2 * RTB)])
            pending = None
            for qt in range(QT):
                # prefetch piece inputs BEFORE the attention emission so these
                # DMAs aren't queued behind the chunk's collective-dependent
                # traffic
                pieces = b1_pieces[qt * 3:(qt + 1) * 3]
                xs = [prefetch_x(ti, rt) for ti, rt in pieces]
                coll, nxt = emit_attn_qt(0, qt)
                for (ti, rt), x_sb in zip(pieces, xs):
                    emit_proj_tile(ti, rt, x_sb)
                # previous chunk's outproj first (its lh waits on an already-
                # finished collective), then trigger this chunk's collective
                if pending is not None:
                    pending()
                pending = nxt
                if coll is not None:
                    coll()
            # b1 vtrans burst overlaps the chunk-1 collective latency
            for ct in range(S // 128, 2 * (S // 128)):
                emit_vtrans(ct)
            if pending is not None:
                pending()
                pending = None
            for qt in range(QT):
                coll, nxt = emit_attn_qt(1, qt)
                if pending is not None:
                    pending()
                pending = nxt
                if coll is not None:
                    coll()
            if pending is not None:
                pending()

    nc.compile()
    return nc


_PROGRAM = None


def _get_program():
    global _PROGRAM
    if _PROGRAM is None:
        _PROGRAM = _build_program()
    return _PROGRAM


def _host_prep(q, k, v, Wq, Wk, Wv, Wp, bp):
    """Build the 8 per-core input maps."""
    rr = lambda a: np.ascontiguousarray(a, dtype=np.float32).astype(np.float16)
    xqT = rr(q.reshape(R, D).T)
    xkT = rr(k.reshape(R, D).T)
    xvT = rr(v.reshape(R, D).T)

    pl = _perm_local()
    perm_global = np.concatenate([128 * c + pl for c in range(NC)])
    wpT = rr(np.ascontiguousarray(Wp.T[perm_global, :]))

    # trig tables
    half = D // 2
    pos = np.arange(S, dtype=np.float64)
    theta = 1.0 / (10000.0 ** (2.0 * np.arange(half, dtype=np.float64) / D))
    ang = pos[:, None] * theta[None, :]          # [S, half]
    cosf = np.cos(ang).astype(FP)                # [S, half]
    sinf = np.sin(ang).astype(FP)

    sw = np.zeros((128, 128), np.float16)
    for m in range(128):
        p = (m + 32) % 64 + 64 * (m // 64)
        sw[p, m] = 1.0
    ident = np.eye(128, dtype=np.float16)
    ones = np.ones((128, 64), np.float16)
    bias = np.broadcast_to(bp.astype(FP), (128, D)).copy()

    in_maps = []
    for c in range(NC):
        cols = 128 * c + pl
        wq_c = rr(np.ascontiguousarray(Wq[cols, :].T))
        wk_c = rr(np.ascontiguousarray(Wk[cols, :].T))
        wv_c = rr(np.ascontiguousarray(Wv[cols, :].T))
        # pair index per partition p (see _perm_local ordering)
        j = np.empty(128, np.int64)
        j[0:32] = 64 * c + np.arange(32)
        j[32:64] = 64 * c + np.arange(32)
        j[64:96] = 64 * c + 32 + np.arange(32)
        j[96:128] = 64 * c + 32 + np.arange(32)
        cs1 = cosf[:, j].T                        # [128, S]
        sn1 = sinf[:, j].T.copy()
        sn1[0:32] *= -1.0
        sn1[64:96] *= -1.0
        cs = np.tile(cs1, (1, B)).astype(np.float16)      # [128, R]
        sn = np.tile(sn1, (1, B)).astype(np.float16)
        in_maps.append({
            "xq": xqT, "xk": xkT, "xv": xvT,
            "wq": wq_c, "wk": wk_c, "wv": wv_c,
            "wp": wpT, "cs": cs, "sn": sn,
            "sw": sw, "ident": ident, "ones": ones, "bias": bias,
        })
    return in_maps


def run(inputs, trace=False, trace_cores=None):
    nc = _get_program()
    in_maps = _host_prep(**inputs)
    res = run_bass_kernel_spmd(
        nc, in_maps, core_ids=list(range(NC)), trace=trace,
        trace_cores=trace_cores,
    )
    outs = np.stack([res.results[c]["out"] for c in range(NC)])  # [c, 512, D]
    # local row (128p + 64g' + i) on core c == global row 512*(2p+g') + 64c + i
    lo = outs.reshape(NC, NC, 64, D)              # [core, (2p,g'), i, D]
    full = lo.transpose(1, 0, 2, 3).reshape(B, S, D)
    return full, res


def kernel(**inputs) -> np.ndarray:
    trace = bool(int(os.environ.get("TRN_TRACE", "0")))
    full, res = run(inputs, trace=trace)
    if trace and res.exec_time_ns is not None:
        print(f"HW exec time: {res.exec_time_ns} ns")
    return full


# revision 20
# speedup vs baseline: 1.3254x; 1.0628x over previous
"""Multi-head attention (RoPE on k/v) Bass kernel for 8 TRN2 NeuronCores.

Sharding: tensor-parallel over heads (2 heads/core, both batches) for the
QKV projections + attention; one AllToAll redistributes context to a
row-sharded output projection. All matmuls fp16.

v2: scores matmul pairs run concurrently in disjoint PE row-groups
(tile_position (0,0)/(64,0)); softmax exp is split across the Scalar
engine (exact LUT exp) and Vector/GpSimd (one-instruction Schraudolph
exp: round(A*s+B) -> int16, bit-identical to an fp16 exponential
approximation, max rel err ~2%).

Self-contained: hardcodes shapes from the problem spec.
"""
import os
import sys
import types

import numpy as np


def _install_ntff_hook():
    """antenv.axon_hooks is missing from this image; synthesize it so
    run_bass_kernel_spmd(trace=True) works. Harmless when trace=False."""
    if "antenv.axon_hooks" in sys.modules:
        return
    try:
        from trn_agent_boot.trn_boot import _ntff_profile_via_ctypes

        hook = _ntff_profile_via_ctypes("/opt/axon/libaxon_pjrt.so")
    except Exception:
        hook = None
    mod = types.ModuleType("antenv.axon_hooks")
    mod._hook = hook
    mod.get_axon_ntff_profile_hook = lambda: mod._hook
    mod.set_axon_ntff_profile_hook = lambda h: setattr(mod, "_hook", h)
    sys.modules["antenv.axon_hooks"] = mod
    try:
        import antenv

        antenv.axon_hooks = mod
    except Exception:
        pass


_install_ntff_hook()

import concourse.bass as bass  # noqa: E402
import concourse.mybir as mybir  # noqa: E402
import concourse.tile as tile  # noqa: E402
from concourse import bacc  # noqa: E402
from concourse.bass import ds  # noqa: E402
from concourse.bass_utils import run_bass_kernel_spmd  # noqa: E402

B, S, D, H = 2, 2048, 1024, 16
R = B * S              # 4096 flattened rows
NC = 8                 # cores
HPC = H // NC          # 2 heads per core
CW = D // NC           # 128 ctx cols per core
DH = D // H            # 64 head dim
RW = R // NC           # 512 output rows per core
F32 = mybir.dt.float32
F16 = mybir.dt.float16
I16 = mybir.dt.int16
FP = np.float32

# Schraudolph fp16 exp constants: i16 = round(raw_score * A + B);
# bitcast(i16) ~= exp(raw_score / 8).
A_SCH = float(1024.0 * np.log2(np.e) / 8.0)
B_SCH = float(15.0 * 1024.0 - 0.043677448 * 1024.0)

# exp engine per kt (16 key tiles per q tile): a=Scalar(exact LUT),
# v=Vector(Schraudolph).  GpSimd cannot read PSUM, so it gets no slots.
EXP_ENG = "aavavaavavaavava"


def _perm_local() -> np.ndarray:
    """Within a core's 128-col slice: [A-evens, A-odds, B-evens, B-odds]."""
    a_ev = np.arange(0, 64, 2)
    a_od = np.arange(1, 64, 2)
    return np.concatenate([a_ev, a_od, 64 + a_ev, 64 + a_od])


def _build_program():
    nc = bacc.Bacc("TRN2", target_bir_lowering=False, debug=False, num_devices=NC)

    # ---- external I/O ----
    xq_d = nc.dram_tensor("xq", [D, R], F16, kind="ExternalInput").ap()
    xk_d = nc.dram_tensor("xk", [D, R], F16, kind="ExternalInput").ap()
    xv_d = nc.dram_tensor("xv", [D, R], F16, kind="ExternalInput").ap()
    wq_d = nc.dram_tensor("wq", [D, CW], F16, kind="ExternalInput").ap()
    wk_d = nc.dram_tensor("wk", [D, CW], F16, kind="ExternalInput").ap()
    wv_d = nc.dram_tensor("wv", [D, CW], F16, kind="ExternalInput").ap()
    wp_d = nc.dram_tensor("wp", [D, D], F16, kind="ExternalInput").ap()
    cs_d = nc.dram_tensor("cs", [128, R], F16, kind="ExternalInput").ap()
    sn_d = nc.dram_tensor("sn", [128, R], F16, kind="ExternalInput").ap()
    sw_d = nc.dram_tensor("sw", [128, 128], F16, kind="ExternalInput").ap()
    id_d = nc.dram_tensor("ident", [128, 128], F16, kind="ExternalInput").ap()
    on_d = nc.dram_tensor("ones", [128, 64], F16, kind="ExternalInput").ap()
    bi_d = nc.dram_tensor("bias", [128, D], F32, kind="ExternalInput").ap()
    out_d = nc.dram_tensor("out", [RW, D], F32, kind="ExternalOutput").ap()

    KT = D // 128   # 8 contraction tiles for projections
    RT = R // 512   # 8 row tiles
    QT = S // 512   # 4 q tiles per batch
    ST = S // 128   # 16 k tiles per batch

    with tile.TileContext(nc) as tc:
        with (
            tc.tile_pool(name="const", bufs=1) as const,
            tc.tile_pool(name="persist", bufs=1) as persist,
            tc.tile_pool(name="dram", bufs=1, space="DRAM") as dram,
            tc.tile_pool(name="wqkv", bufs=1) as wpool,
            tc.tile_pool(name="trig", bufs=1) as trig,
            tc.tile_pool(name="xin", bufs=3) as xin,
            tc.tile_pool(name="vtmp", bufs=1) as vtmp,
            tc.tile_pool(name="rope", bufs=3) as rp,
            tc.tile_pool(name="epool", bufs=1) as ep,
            tc.tile_pool(name="norm", bufs=1) as npl,
            tc.tile_pool(name="oio", bufs=2) as oio,
            tc.tile_pool(name="pp", bufs=1, space="PSUM") as pp,
            tc.tile_pool(name="spsum", bufs=1, space="PSUM") as sp,
            tc.tile_pool(name="cpsum", bufs=1, space="PSUM") as cp,
        ):
            # ---- constants / weights ----
            # Only wk + sw are DMA'd up front; the rest are issued at their
            # first use point so the opening k-proj isn't DMA-starved.
            wk_sb = wpool.tile([128, KT, CW], F16)
            nc.sync.dma_start(wk_sb[:], wk_d.rearrange("(kt p) m -> p kt m", p=128))
            sw_sb = const.tile([128, 128], F16)
            nc.sync.dma_start(sw_sb[:], sw_d[:])
            id_sb = const.tile([128, 128], F16)
            on_sb = const.tile([128, 64], F16)
            wv_sb = wpool.tile([128, KT, CW], F16)
            wq_sb = wpool.tile([128, KT, CW], F16)
            cs_sb = trig.tile([128, R], F16)
            sn_sb = trig.tile([128, R], F16)
            bi_sb = const.tile([128, D], F32)
            wp_sb = const.tile([128, KT, D], F16)

            qpT = persist.tile([128, R], F16)
            kpT = persist.tile([128, R], F16)
            vaug = persist.tile([128, R // 128, 256], F16)
            vpT = vtmp.tile([128, R], F16)

            def emit_vaug_init():
                nc.vector.tensor_copy(
                    vaug[:, :, 64:128],
                    on_sb[:, :, None].rearrange("p o n -> p n o").to_broadcast((128, R // 128, 64)),
                )
                nc.vector.tensor_copy(
                    vaug[:, :, 192:256],
                    on_sb[:, :, None].rearrange("p o n -> p n o").to_broadcast((128, R // 128, 64)),
                )

            _chunk_rows = [128, 128, 128, 64, 64]
            a2a_ins = [dram.tile([NC, 128, _chunk_rows[p]], F16,
                                 name=f"a2ain{p}", tag=f"a2ain{p}")
                       for p in range(5)]
            a2a_outs = [dram.tile([NC, 128, _chunk_rows[p]], F16,
                                  name=f"a2aout{p}", tag=f"a2aout{p}")
                        for p in range(5)]

            RTB = RT // B     # 4 row-tiles per batch
            CHUNKS = [(0, 2), (2, 4), (4, 6), (6, 7), (7, 8)]  # groups per A2A

            def prefetch_x(ti, rt):
                xd = (xk_d, xv_d, xq_d)[ti]
                rsl = ds(rt * 512, 512)
                x_sb = xin.tile([128, KT, 512], F16, tag="xin", name="x_sb")
                # split so the first contraction tiles land (and unblock the
                # matmuls) before the whole slice has arrived
                xr = xd.rearrange("(kt p) r -> p kt r", p=128)
                nc.sync.dma_start(x_sb[:, 0:2], xr[:, 0:2, rsl])
                nc.sync.dma_start(x_sb[:, 2:KT], xr[:, 2:KT, rsl])
                return x_sb

            def emit_proj_tile(ti, rt, x_sb=None):
                xd, wsb, dest, do_rope = (
                    (xk_d, wk_sb, kpT, True),
                    (xv_d, wv_sb, vpT, True),
                    (xq_d, wq_sb, qpT, False),
                )[ti]
                rsl = ds(rt * 512, 512)
                if x_sb is None:
                    x_sb = prefetch_x(ti, rt)
                ps = pp.tile([128, 512], F32, tag="proj", bufs=2, name="ps")
                for kt in range(KT):
                    nc.tensor.matmul(
                        ps[:], wsb[:, kt], x_sb[:, kt],
                        start=(kt == 0), stop=(kt == KT - 1),
                    )
                if not do_rope:
                    nc.vector.tensor_copy(dest[:, rsl], ps[:])
                else:
                    raw = rp.tile([128, 512], F16, tag="raw", name="raw")
                    nc.scalar.activation(raw[:], ps[:],
                                         mybir.ActivationFunctionType.Copy)
                    sps = pp.tile([128, 512], F32, tag="proj", bufs=2, name="sps")
                    nc.tensor.matmul(sps[:], sw_sb[:], raw[:], start=True, stop=True)
                    t1 = rp.tile([128, 512], F32, tag="t1", name="t1")
                    nc.vector.tensor_tensor(
                        t1[:], raw[:], cs_sb[:, rsl], mybir.AluOpType.mult)
                    t2 = rp.tile([128, 512], F32, tag="t2", name="t2")
                    nc.vector.tensor_tensor(
                        t2[:], sps[:], sn_sb[:, rsl], mybir.AluOpType.mult)
                    nc.vector.tensor_tensor(
                        dest[:, rsl], t1[:], t2[:], mybir.AluOpType.add)

            def emit_vtrans(ct):
                tpx = pp.tile([128, 512], F16, tag="proj", bufs=2, name="tpx")
                tp = tpx[:, 0:128]
                nc.tensor.transpose(tp, vpT[:, ds(ct * 128, 128)], id_sb[:])
                nc.vector.tensor_copy(vaug[:, ct, 0:64], tp[:, 0:64])
                nc.vector.tensor_copy(vaug[:, ct, 128:192], tp[:, 64:128])

            def emit_attn_qt(bb, qt):
                """Returns a deferred-outproj closure (or None): caller emits
                it after queueing more PE work, so the output-projection
                matmuls (which wait on the AllToAll) don't head-of-line-block
                the PE FIFO."""
                qsl = ds(bb * S + qt * 512, 512)
                # E[:, kt, 0, :] = head A weights, [:, kt, 1, :] = head B
                E = ep.tile([128, ST, 2, 512], F16, tag="E", bufs=2, name="E")
                cA = cp.tile([128, 512], F32, tag="cA", bufs=1, name="cA")
                cB = cp.tile([128, 512], F32, tag="cB", bufs=1, name="cB")

                def ctx_kt(kt):
                    ct = bb * ST + kt
                    nc.tensor.matmul(cA[:], vaug[:, ct, 0:128], E[:, kt, 0],
                                     start=(kt == 0), stop=(kt == ST - 1))
                    nc.tensor.matmul(cB[:], vaug[:, ct, 128:256], E[:, kt, 1],
                                     start=(kt == 0), stop=(kt == ST - 1))

                for kt in range(ST):
                    ksl = ds(bb * S + kt * 128, 128)
                    ps = sp.tile([128, 2, 512], F32, tag="sc", bufs=2,
                                 name="ps_sc")
                    nc.tensor.matmul(
                        ps[:, 0], kpT[0:64, ksl], qpT[0:64, qsl],
                        start=True, stop=True, tile_position=(0, 0))
                    nc.tensor.matmul(
                        ps[:, 1], kpT[64:128, ksl], qpT[64:128, qsl],
                        start=True, stop=True, tile_position=(64, 0))
                    eng = EXP_ENG[kt]
                    if eng == "a":
                        nc.scalar.activation(
                            E[:, kt], ps[:],
                            mybir.ActivationFunctionType.Exp, scale=0.125)
                    elif eng == "v":
                        nc.vector.tensor_scalar(
                            E[:, kt].bitcast(I16), ps[:], A_SCH, B_SCH,
                            mybir.AluOpType.mult, mybir.AluOpType.add)
                    else:
                        nc.gpsimd.tensor_scalar(
                            E[:, kt].bitcast(I16), ps[:], A_SCH, B_SCH,
                            mybir.AluOpType.mult, mybir.AluOpType.add)
                    if kt >= 2:
                        ctx_kt(kt - 2)
                ctx_kt(ST - 2)
                ctx_kt(ST - 1)

                zhi = npl.tile([128, 1024], F32, tag="zhi", name="zhi")
                nc.vector.tensor_copy(zhi[64:128, 0:512], cA[64:128])
                nc.vector.tensor_copy(zhi[64:128, 512:1024], cB[64:128])
                zlo = npl.tile([64, 1024], F32, tag="zlo", name="zlo")
                nc.sync.dma_start(zlo[:], zhi[64:128, :])
                zr = npl.tile([64, 1024], F32, tag="zr", name="zr")
                nc.vector.reciprocal_approx_fast(zr[:], zlo[:])
                ctxA = npl.tile([64, 512], F16, tag="ctxA", name="ctxA")
                nc.vector.tensor_tensor(
                    ctxA[:], cA[0:64], zr[:, 0:512], mybir.AluOpType.mult)
                ctxB = npl.tile([64, 512], F16, tag="ctxB", name="ctxB")
                nc.vector.tensor_tensor(
                    ctxB[:], cB[0:64], zr[:, 512:1024], mybir.AluOpType.mult)
                shard = bb * QT + qt
                ck = next(i for i, (a, b) in enumerate(CHUNKS) if a <= shard < b)
                a, b = CHUNKS[ck]
                rsl2 = ds((shard - a) * 64, 64)
                nc.sync.dma_start(
                    a2a_ins[ck][:, 0:64, rsl2].rearrange("j p r -> p j r"),
                    ctxA[:].rearrange("p (j r) -> p j r", j=NC))
                nc.sync.dma_start(
                    a2a_ins[ck][:, 64:128, rsl2].rearrange("j p r -> p j r"),
                    ctxB[:].rearrange("p (j r) -> p j r", j=NC))
                if shard != b - 1:
                    return None, None
                nrows = (b - a) * 64

                def do_collective():
                    nc.gpsimd.collective_compute(
                        "AllToAll",
                        mybir.AluOpType.bypass,
                        replica_groups=[list(range(NC))],
                        ins=[a2a_ins[ck].opt()],
                        outs=[a2a_outs[ck].opt()],
                    )

                def do_outproj():
                    lh = oio.tile([128, NC, 128], F16, tag="lh", name="lh")
                    # gpsimd queue (not sync): this DMA waits on the
                    # collective; on the sync queue it would FIFO-block later
                    # x prefetches / staging
                    nc.gpsimd.dma_start(
                        lh[:, :, 0:nrows],
                        a2a_outs[ck][:].rearrange("j p r -> p j r"),
                    )
                    for oc in range(2):
                        po = pp.tile([128, 512], F32, tag="proj", bufs=2,
                                     name="po")
                        for j in range(NC):
                            nc.tensor.matmul(
                                po[0:nrows], lh[:, j, 0:nrows],
                                wp_sb[:, j, ds(oc * 512, 512)],
                                start=(j == 0), stop=(j == NC - 1))
                        ob = oio.tile([128, 512], F32, tag="ob", name="ob")
                        nc.vector.tensor_tensor(
                            ob[0:nrows], po[0:nrows], bi_sb[0:nrows, ds(oc * 512, 512)],
                            mybir.AluOpType.add)
                        nc.sync.dma_start(
                            out_d[ds(a * 64, nrows), ds(oc * 512, 512)], ob[0:nrows])

                return do_collective, do_outproj

            # ---- batch 0 projections ----
            # DMA order matters at startup: the first k-proj tile needs only
            # wk + x(0,0) (+ trig/sw for its rope) — everything else later.
            x00 = prefetch_x(0, 0)
            x01 = prefetch_x(0, 1)
            nc.sync.dma_start(cs_sb[:], cs_d[:])
            nc.sync.dma_start(sn_sb[:], sn_d[:])
            emit_proj_tile(0, 0, x00)
            emit_proj_tile(0, 1, x01)
            nc.sync.dma_start(wv_sb[:], wv_d.rearrange("(kt p) m -> p kt m", p=128))
            nc.sync.dma_start(wq_sb[:], wq_d.rearrange("(kt p) m -> p kt m", p=128))
            nc.sync.dma_start(id_sb[:], id_d[:])
            nc.sync.dma_start(on_sb[:], on_d[:])
            for rt in range(2, RTB):
                emit_proj_tile(0, rt)
            emit_vaug_init()
            for ti in (1, 2):
                for rt in range(0, RTB):
                    emit_proj_tile(ti, rt)
            nc.sync.dma_start(bi_sb[:], bi_d[:])
            nc.sync.dma_start(wp_sb[:], wp_d.rearrange("(kt p) o -> p kt o", p=128))
            for ct in range(0, S // 128):
                emit_vtrans(ct)
            # ---- batch 0 attention; batch 1 projections interleaved ----
            # piece order: k then v (so b1 vtrans can run right after qt3),
            # then q (only needed once b1 attention starts)
            b1_pieces = ([(0, rt) for rt in range(RTB, 2 * RTB)]
                         + [(1, rt) for rt in range(RTB, 2 * RTB)]
                         + [(2, rt) for rt in range(RTB, 2 * RTB)])
            # deferred output projections: flush a chunk's po two scheduling
            # steps after its collective fired, so the po matmuls never sit
            # in the PE queue ahead of work that could overlap the transfer
            deferred = []

            def flush_deferred(now, horizon=2):
                keep = []
                for step, fn in deferred:
                    if now - step >= horizon:
                        fn()
                    else:
                        keep.append((step, fn))
                deferred[:] = keep

            step = 0
            for qt in range(QT):
                # prefetch piece inputs BEFORE the attention emission so these
                # DMAs aren't queued behind the chunk's collective-dependent
                # traffic
                pieces = b1_pieces[qt * 3:(qt + 1) * 3]
                xs = [prefetch_x(ti, rt) for ti, rt in pieces]
                coll, nxt = emit_attn_qt(0, qt)
                for (ti, rt), x_sb in zip(pieces, xs):
                    emit_proj_tile(ti, rt, x_sb)
                flush_deferred(step)
                if coll is not None:
                    coll()
                    deferred.append((step, nxt))
                step += 1
            # b1 vtrans burst overlaps the chunk-1 collective latency
            for ct in range(S // 128, 2 * (S // 128)):
                emit_vtrans(ct)
            flush_deferred(step)
            step += 1
            for qt in range(QT):
                coll, nxt = emit_attn_qt(1, qt)
                flush_deferred(step)
                if coll is not None:
                    coll()
                    deferred.append((step, nxt))
                step += 1
            flush_deferred(step + 10)

    nc.compile()
    return nc


_PROGRAM = None


def _get_program():
    global _PROGRAM
    if _PROGRAM is None:
        _PROGRAM = _build_program()
    return _PROGRAM


def _host_prep(q, k, v, Wq, Wk, Wv, Wp, bp):
    """Build the 8 per-core input maps."""
    rr = lambda a: np.ascontiguousarray(a, dtype=np.float32).astype(np.float16)
    xqT = rr(q.reshape(R, D).T)
    xkT = rr(k.reshape(R, D).T)
    xvT = rr(v.reshape(R, D).T)

    pl = _perm_local()
    perm_global = np.concatenate([128 * c + pl for c in range(NC)])
    wpT = rr(np.ascontiguousarray(Wp.T[perm_global, :]))

    # trig tables
    half = D // 2
    pos = np.arange(S, dtype=np.float64)
    theta = 1.0 / (10000.0 ** (2.0 * np.arange(half, dtype=np.float64) / D))
    ang = pos[:, None] * theta[None, :]          # [S, half]
    cosf = np.cos(ang).astype(FP)                # [S, half]
    sinf = np.sin(ang).astype(FP)

    sw = np.zeros((128, 128), np.float16)
    for m in range(128):
        p = (m + 32) % 64 + 64 * (m // 64)
        sw[p, m] = 1.0
    ident = np.eye(128, dtype=np.float16)
    ones = np.ones((128, 64), np.float16)
    bias = np.broadcast_to(bp.astype(FP), (128, D)).copy()

    in_maps = []
    for c in range(NC):
        cols = 128 * c + pl
        wq_c = rr(np.ascontiguousarray(Wq[cols, :].T))
        wk_c = rr(np.ascontiguousarray(Wk[cols, :].T))
        wv_c = rr(np.ascontiguousarray(Wv[cols, :].T))
        # pair index per partition p (see _perm_local ordering)
        j = np.empty(128, np.int64)
        j[0:32] = 64 * c + np.arange(32)
        j[32:64] = 64 * c + np.arange(32)
        j[64:96] = 64 * c + 32 + np.arange(32)
        j[96:128] = 64 * c + 32 + np.arange(32)
        cs1 = cosf[:, j].T                        # [128, S]
        sn1 = sinf[:, j].T.copy()
        sn1[0:32] *= -1.0
        sn1[64:96] *= -1.0
        cs = np.tile(cs1, (1, B)).astype(np.float16)      # [128, R]
        sn = np.tile(sn1, (1, B)).astype(np.float16)
        in_maps.append({
            "xq": xqT, "xk": xkT, "xv": xvT,
            "wq": wq_c, "wk": wk_c, "wv": wv_c,
            "wp": wpT, "cs": cs, "sn": sn,
            "sw": sw, "ident": ident, "ones": ones, "bias": bias,
        })
    return in_maps


def run(inputs, trace=False, trace_cores=None):
    nc = _get_program()
    in_maps = _host_prep(**inputs)
    res = run_bass_kernel_spmd(
        nc, in_maps, core_ids=list(range(NC)), trace=trace,
        trace_cores=trace_cores,
    )
    outs = np.stack([res.results[c]["out"] for c in range(NC)])  # [c, 512, D]
    # local row (128p + 64g' + i) on core c == global row 512*(2p+g') + 64c + i
    lo = outs.reshape(NC, NC, 64, D)              # [core, (2p,g'), i, D]
    full = lo.transpose(1, 0, 2, 3).reshape(B, S, D)
    return full, res


def kernel(**inputs) -> np.ndarray:
    trace = bool(int(os.environ.get("TRN_TRACE", "0")))
    full, res = run(inputs, trace=trace)
    if trace and res.exec_time_ns is not None:
        print(f"HW exec time: {res.exec_time_ns} ns")
    return full
